# revision 1
# baseline (speedup 1.0000x reference)
"""DifferentialCausalAttention on 8 Trainium2 NeuronCores.

Sharding: 8 cores = 2 batches x 4 head-groups (tensor-parallel over heads).
Core c handles batch b = c // 4 and head-group g = c % 4:
  - query heads 8g..8g+7 (4 pairs), kv heads 4g..4g+3, lambda cols 4g..4g+3
  - W_O rows 512g..512g+511 -> partial output, host-summed over the 4 groups.

All matmuls run in float32r (full-rate fp32 mode on the PE).
Layouts on device: Q^T/K^T as [dh, L] (dh on partitions), V as [L, d],
attention computed transposed (S^T = [k, q]) so no P-transposes are needed.
"""
import os
from contextlib import ExitStack

import ml_dtypes
import numpy as np

import concourse.bass as bass
import concourse.mybir as mybir
import concourse.tile as tile
from concourse import bacc
from concourse.bass_utils import run_bass_kernel_spmd

F32 = mybir.dt.float32
F32R = mybir.dt.float32r
BF16 = mybir.dt.bfloat16

# dtype config: bf16 halves DMA and makes weight loads fast/overlappable,
# fp32r is ~20x more accurate. Toggles for experimentation.
ATT_BF16 = os.environ.get("ATT_BF16", "1") == "1"   # phase-2 S/ctx/rs operands
P1_BF16 = os.environ.get("P1_BF16", "1") == "1"     # phase-1 projection operands
P3_BF16 = os.environ.get("P3_BF16", "1") == "1"     # phase-3 Wo/diffT
DT_ATT = BF16 if ATT_BF16 else F32R
DT_P1 = BF16 if P1_BF16 else F32R
DT_P3 = BF16 if P3_BF16 else F32R
NP_P1 = ml_dtypes.bfloat16 if P1_BF16 else np.float32
NP_ATT = ml_dtypes.bfloat16 if ATT_BF16 else np.float32
NP_P3 = ml_dtypes.bfloat16 if P3_BF16 else np.float32

B, L, D, NH = 2, 2048, 2048, 16
DH = D // NH            # 128
G = 4                   # head groups (cores per batch)
NKV = NH // G           # kv heads per core = 4
NQ = 2 * NKV            # query heads per core = 8
CQK = NQ * DH + NKV * DH  # 1536 projection cols (Q then K)
CT = CQK // 128         # 12 column tiles (0-7 Q heads, 8-11 K heads)
DC = D // 128           # 16 contraction chunks
LCH = L // 512          # 4 L-chunks
LT = L // 128           # 16 L-tiles / q-tiles
SCALE = 1.0 / float(np.sqrt(DH))
ROPE_BASE = 10000.0


def build_kernel() -> bacc.Bacc:
    nc = bacc.Bacc("TRN2", target_bir_lowering=False, debug=False)

    xT = nc.dram_tensor("xT", [D, L], DT_P1, kind="ExternalInput")
    Wqk = nc.dram_tensor("Wqk", [D, CQK], DT_P1, kind="ExternalInput")
    Wv = nc.dram_tensor("Wv", [D, NKV * DH], DT_P1, kind="ExternalInput")
    Wl = nc.dram_tensor("Wl", [D, 128], DT_P1, kind="ExternalInput")
    blv = nc.dram_tensor("blv", [128, 1], F32, kind="ExternalInput")
    Wo = nc.dram_tensor("Wo", [NKV * DH, D], DT_P3, kind="ExternalInput")
    cosT = nc.dram_tensor("cosT", [DH, L], F32, kind="ExternalInput")
    sinTs = nc.dram_tensor("sinTs", [DH, L], F32, kind="ExternalInput")
    maskT = nc.dram_tensor("maskT", [128, 256], DT_ATT, kind="ExternalInput")
    onesin = nc.dram_tensor("onesin", [128, 128], F32R, kind="ExternalInput")
    outT = nc.dram_tensor("outT", [D, L], F32, kind="ExternalOutput")
    dbg = None
    if os.environ.get("KDBG") == "1":
        dbg = nc.dram_tensor("dbg", [20, L], F32, kind="ExternalOutput")

    with ExitStack() as ctx:
        tc = ctx.enter_context(tile.TileContext(nc))

        persist = ctx.enter_context(tc.tile_pool(name="persist", bufs=1))
        dram = ctx.enter_context(tc.tile_pool(name="dram", bufs=1, space="DRAM"))

        # ---- persistent tiles (loads deferred so phase-1 tiles go first) ----
        mask_sb = persist.tile([128, 256], DT_ATT)
        ones_sb = persist.tile([128, 128], F32R)
        ones_att = persist.tile([128, 1], DT_ATT)
        bl_sb = persist.tile([128, 1], F32)
        wo_sb = persist.tile([128, NKV, D], DT_P3)
        lam_sb = persist.tile([NKV, L], F32)          # sigmoid(x@Wl+bl), row per kv head
        diffT = persist.tile([128, NKV, L], DT_P3)     # (ctx0 - lam*ctx1)^T per head

        # DRAM scratch between phases
        qkT_d = dram.tile([CT, 128, L], DT_ATT)         # Q^T/K^T after RoPE
        v_d = dram.tile([L, NKV * DH], DT_ATT)          # V in [L, d] layout

        # ================= Phase 1: projections + RoPE =================
        with tc.tile_pool(name="ph1", bufs=1) as ph1, \
                tc.tile_pool(name="ps1", bufs=1, space="PSUM") as ps1:
            xTr = xT.rearrange("(dc p) l -> p dc l", p=128)
            wqkr = Wqk.rearrange("(dc p) c -> p dc c", p=128)
            wv_sb = ph1.tile([128, DC, NKV * DH], DT_P1)
            wl_sb = ph1.tile([128, DC, 128], DT_P1)

            for lch in range(LCH):
                ls = slice(lch * 512, (lch + 1) * 512)
                xs = ph1.tile([128, DC, 512], DT_P1, name="xs", tag="xs", bufs=2)
                for dc in range(DC):
                    nc.sync.dma_start(xs[:, dc, :], xTr[:, dc, ls])
                cos_sb = ph1.tile([128, 512], F32, name="cos_sb", tag="cos", bufs=2)
                sin_sb = ph1.tile([128, 512], F32, name="sin_sb", tag="sin", bufs=2)
                if os.environ.get("NO_TAB") != "1":
                    nc.sync.dma_start(cos_sb[:], cosT[:, ls])
                    nc.sync.dma_start(sin_sb[:], sinTs[:, ls])

                # --- Q^T / K^T column tiles + RoPE ---
                for ct in range(CT):
                    wt = ph1.tile([128, DC, 128], DT_P1, name="wt", tag="wt", bufs=3)
                    nc.sync.dma_start(wt[:], wqkr[:, :, ct * 128:(ct + 1) * 128])
                    qk_ps = ps1.tile([128, 512], F32, name="qk_ps", tag="mm512", bufs=4)
                    for dc in range(DC):
                        nc.tensor.matmul(
                            qk_ps[:], wt[:, dc, :], xs[:, dc, :],
                            start=(dc == 0), stop=(dc == DC - 1),
                        )
                    # RoPE: qr = qk*cos + rot(qk)*sin_signed
                    qkr_sb = ph1.tile([128, 512], DT_ATT, name="qkr_sb", tag="qkr", bufs=3)
                    if os.environ.get("NO_ROPE") == "1":
                        nc.scalar.copy(qkr_sb[:], qk_ps[:])
                    else:
                        rot = ph1.tile([128, 512], F32, name="rot", tag="rot", bufs=2)
                        nc.scalar.copy(rot[0:64, :], qk_ps[64:128, :])
                        nc.scalar.copy(rot[64:128, :], qk_ps[0:64, :])
                        t1 = ph1.tile([128, 512], F32, name="t1", tag="t1", bufs=2)
                        nc.vector.tensor_mul(t1[:], qk_ps[:], cos_sb[:])
                        t2 = ph1.tile([128, 512], F32, name="t2", tag="t2", bufs=2)
                        nc.vector.tensor_mul(t2[:], rot[:], sin_sb[:])
                        nc.vector.tensor_add(qkr_sb[:], t1[:], t2[:])
                    nc.sync.dma_start(qkT_d[ct, :, ls], qkr_sb[:])
                    if lch == 0 and ct == 0:
                        # big/aux loads ride behind the first column tile
                        nc.sync.dma_start(
                            wv_sb[:], Wv.rearrange("(dc p) c -> p dc c", p=128)
                        )
                        nc.sync.dma_start(
                            wl_sb[:], Wl.rearrange("(dc p) c -> p dc c", p=128)
                        )
                        nc.sync.dma_start(bl_sb[:], blv[:, :])
                    if dbg is not None and ct == 0 and os.environ.get("NO_ROPE") != "1":
                        for di, srcap in enumerate(
                            (qk_ps[0:1, :], cos_sb[0:1, :], sin_sb[0:1, :], rot[0:1, :])
                        ):
                            db = ph1.tile([1, 512], F32, name="db", tag=f"db{di}", bufs=2)
                            nc.vector.tensor_copy(db[:], srcap)
                            nc.sync.dma_start(dbg[NKV + 2 + di:NKV + 3 + di, ls], db[:])

                # --- V tiles ---
                for lt in ([] if os.environ.get("NO_V") == "1" else range(4)):
                    v_ps = ps1.tile([128, 512], F32, name="v_ps", tag="mm512", bufs=4)
                    for dc in range(DC):
                        nc.tensor.matmul(
                            v_ps[:], xs[:, dc, lt * 128:(lt + 1) * 128], wv_sb[:, dc, :],
                            start=(dc == 0), stop=(dc == DC - 1),
                        )
                    v_sb = ph1.tile([128, 512], DT_ATT, name="v_sb", tag="v_sb", bufs=2)
                    nc.scalar.copy(v_sb[:], v_ps[:])
                    nc.sync.dma_start(
                        v_d[lch * 512 + lt * 128: lch * 512 + (lt + 1) * 128, :], v_sb[:]
                    )

                # --- lambda ---
                if os.environ.get("NO_LAM") == "1":
                    continue
                lam_ps = ps1.tile([128, 512], F32, name="lam_ps", tag="mm512", bufs=4)
                for dc in range(DC):
                    nc.tensor.matmul(
                        lam_ps[:], wl_sb[:, dc, :], xs[:, dc, :],
                        start=(dc == 0), stop=(dc == DC - 1),
                    )
                nc.scalar.activation(
                    lam_sb[:, ls], lam_ps[0:NKV, :],
                    mybir.ActivationFunctionType.Sigmoid, bias=bl_sb[0:NKV, 0:1],
                )

        if dbg is not None:
            if os.environ.get("NO_LAM") != "1":
                nc.sync.dma_start(dbg[0:NKV, :], lam_sb[:, :])
            for cti in range(CT):
                nc.sync.dma_start(dbg[NKV + cti:NKV + cti + 1, :], qkT_d[cti, 0:1, :].bitcast(F32))
        trunc = os.environ.get("KTRUNC") == "1"
        # ================= Phase 2: causal attention per head pair =================
        # Two q-tiles (a "superblock": A=2sb, B=2sb+1) are processed at once so
        # every moving operand is 512 wide: columns ordered (qtile, head, l) =
        # [A.h0 | A.h1 | B.h0 | B.h1]. Per k-chunk: one S matmul [128,512], one
        # exp, one ctx matmul, one rowsum matmul. Chunk kc==A is full width but
        # masks its [0:256] half; chunk kc==B covers only [256:512]. ctx/rs for
        # chunk kc are emitted after the S matmul of chunk kc+2 (PE never waits
        # on ACT); the normalization tail is deferred into the next superblock.
        with tc.tile_pool(name="ph2", bufs=1) as ph2, \
                tc.tile_pool(name="ps2", bufs=1, space="PSUM") as ps2:
            v_r = v_d.rearrange("(kc pp) d -> pp kc d", pp=128)
            pend_norm = []

            def emit_block(st):
                ctx_ps, rs_ps, e_sb, j, kc, qtB, off, wid = st
                nc.tensor.matmul(
                    ctx_ps[:, off:off + wid], vp_sb[:, kc, :], e_sb[:, j, off:off + wid],
                    start=(kc == 0), stop=(kc == qtB), skip_group_check=True,
                )
                nc.tensor.matmul(
                    rs_ps[0:1, off:off + wid], ones_att[:, 0:1], e_sb[:, j, off:off + wid],
                    start=(kc == 0), stop=(kc == qtB), skip_group_check=True,
                )

            def emit_norm(st):
                ctx_ps, rs_sb, qtA, p_, lam0_ = st
                recip = ph2.tile([1, 2, 256], F32, name="recip", tag="recip", bufs=2)
                nc.vector.reciprocal_approx_fast(
                    recip.rearrange("p t l -> p (t l)"), rs_sb[:]
                )
                cs = ph2.tile([1, 2, 256], F32R, name="cs", tag="cs", bufs=2)
                nc.vector.tensor_copy(cs[:, :, 0:128], recip[:, :, 0:128])
                nc.vector.tensor_mul(
                    cs[:, :, 128:256], recip[:, :, 128:256],
                    lam0_[:, qtA * 128:(qtA + 2) * 128].rearrange(
                        "p (t l) -> p t l", t=2
                    ),
                )
                b_ps = ps2.tile([128, 512], F32, name="b_ps", tag="bps", bufs=1)
                nc.tensor.matmul(
                    b_ps[:], ones_sb[0:1, :], cs.rearrange("p t l -> p (t l)"),
                    start=True, stop=True,
                )
                b_sb = ph2.tile([128, 2, 256], F32, name="b_sb", tag="bsb", bufs=2)
                nc.vector.tensor_copy(b_sb.rearrange("p t l -> p (t l)"), b_ps[:])
                ctx3 = ctx_ps.rearrange("p (t l) -> p t l", t=2)
                t0 = ph2.tile([128, 2, 128], F32, name="t0", tag="t0", bufs=2)
                nc.vector.tensor_mul(t0[:], ctx3[:, :, 0:128], b_sb[:, :, 0:128])
                t1b = ph2.tile([128, 2, 128], F32, name="t1b", tag="t1b", bufs=2)
                nc.vector.tensor_mul(t1b[:], ctx3[:, :, 128:256], b_sb[:, :, 128:256])
                nc.vector.tensor_sub(
                    diffT[:, p_, qtA * 128:(qtA + 2) * 128],
                    t0.rearrange("p t l -> p (t l)"),
                    t1b.rearrange("p t l -> p (t l)"),
                )

            if not trunc:
                nc.sync.dma_start(mask_sb[:], maskT[:, :])
                nc.sync.dma_start(ones_sb[:], onesin[:, :])
                nc.vector.tensor_copy(ones_att[:], ones_sb[:, 0:1])
                nc.sync.dma_start(wo_sb[:], Wo.rearrange("(p d) o -> d p o", d=128))
            for p in ([] if trunc else range(NKV)):
                # [128, 16(t), 2(h), 128(l)] so superblock slices are contiguous
                qt_sb = ph2.tile([128, LT, 2, 128], DT_ATT, name="qt_sb", tag="qt", bufs=2)
                nc.sync.dma_start(
                    qt_sb[:],
                    qkT_d[2 * p:2 * p + 2, :, :].rearrange(
                        "h p (t l) -> p t h l", t=LT
                    ),
                )
                lam0 = ph2.tile([1, L], F32, name="lam0", tag="lam0", bufs=2)
                nc.gpsimd.dma_start(lam0[:], lam_sb[p:p + 1, :])
                kt_sb = ph2.tile([128, L], DT_ATT, name="kt_sb", tag="kt", bufs=2)
                nc.sync.dma_start(kt_sb[:], qkT_d[NQ + p, :, :])
                vp_sb = ph2.tile([128, LT, 128], DT_ATT, name="vp_sb", tag="vp", bufs=2)
                nc.sync.dma_start(vp_sb[:], v_r[:, :, p * 128:(p + 1) * 128])

                for sb in range(LT // 2):
                    qtA, qtB = 2 * sb, 2 * sb + 1
                    ctx_ps = ps2.tile([128, 512], F32, name="ctx_ps", tag="ctx", bufs=2)
                    rs_ps = ps2.tile([1, 512], F32, name="rs_ps", tag="small", bufs=1)
                    pend = []
                    # chunk groups of 2 sharing one 2-bank S tile; exp per group
                    groups = []
                    kcs = list(range(qtB + 1))
                    for gi in range(0, len(kcs), 2):
                        groups.append(kcs[gi:gi + 2])
                    for gk, grp in enumerate(groups):
                        s_ps = ps2.tile([128, 2, 512], F32, name="s_ps", tag="s2", bufs=2)
                        segs = []
                        for j, kc in enumerate(grp):
                            off, wid = (256, 256) if kc == qtB else (0, 512)
                            rhs = (qt_sb[:, qtA:qtA + 2, :, :] if wid == 512
                                   else qt_sb[:, qtB, :, :])
                            nc.tensor.matmul(
                                s_ps[:, j, off:off + wid],
                                kt_sb[:, kc * 128:(kc + 1) * 128],
                                rhs,
                                start=True, stop=True, skip_group_check=True,
                            )
                            segs.append((j, kc, off, wid))
                        if gk == 0 and len(pend_norm) >= 1:
                            emit_norm(pend_norm.pop(0))  # deferred tail
                        while len(pend) >= 2:
                            emit_block(pend.pop(0))
                        e_sb = ph2.tile([128, 2, 512], DT_ATT, name="e_sb", tag="e", bufs=3)
                        lo = segs[0][2]
                        hi = segs[-1][2] + segs[-1][3]
                        if len(segs) == 2 and segs[0][3] == 512 and segs[1][3] == 512:
                            nc.scalar.activation(
                                e_sb.rearrange("p a b -> p (a b)"),
                                s_ps.rearrange("p a b -> p (a b)"),
                                mybir.ActivationFunctionType.Exp, scale=SCALE,
                            )
                        else:
                            for j, kc, off, wid in segs:
                                nc.scalar.activation(
                                    e_sb[:, j, off:off + wid], s_ps[:, j, off:off + wid],
                                    mybir.ActivationFunctionType.Exp, scale=SCALE,
                                )
                        for j, kc, off, wid in segs:
                            if kc == qtA:
                                nc.vector.tensor_mul(
                                    e_sb[:, j, 0:256], e_sb[:, j, 0:256], mask_sb[:]
                                )
                            elif kc == qtB:
                                nc.vector.tensor_mul(
                                    e_sb[:, j, 256:512], e_sb[:, j, 256:512], mask_sb[:]
                                )
                            pend.append((ctx_ps, rs_ps, e_sb, j, kc, qtB, off, wid))
                    for st in pend:
                        emit_block(st)
                    # eager rowsum copy frees the PSUM bank promptly (DVE)
                    rs_sb = ph2.tile([1, 512], F32, name="rs_sb", tag="rs_sb", bufs=3)
                    nc.vector.tensor_copy(rs_sb[:], rs_ps[:])
                    pend_norm.append((ctx_ps, rs_sb, qtA, p, lam0))
            for st in pend_norm:
                emit_norm(st)

        # ================= Phase 3: output projection =================
        with tc.tile_pool(name="ph3", bufs=1) as ph3, \
                tc.tile_pool(name="ps3", bufs=1, space="PSUM") as ps3:
            for ot in ([] if trunc else range(LT)):
                for qch in range(LCH):
                    o_ps = ps3.tile([128, 512], F32, name="o_ps", tag="mm512", bufs=4)
                    for p in range(NKV):
                        nc.tensor.matmul(
                            o_ps[:],
                            wo_sb[:, p, ot * 128:(ot + 1) * 128],
                            diffT[:, p, qch * 512:(qch + 1) * 512],
                            start=(p == 0), stop=(p == NKV - 1),
                        )
                    o_sb = ph3.tile([128, 512], F32, name="o_sb", tag="osb", bufs=4)
                    nc.scalar.copy(o_sb[:], o_ps[:])
                    nc.sync.dma_start(
                        outT[ot * 128:(ot + 1) * 128, qch * 512:(qch + 1) * 512], o_sb[:]
                    )

    nc.finalize()
    return nc


def _host_tables():
    half = DH // 2
    inv_freq = 1.0 / (ROPE_BASE ** (np.arange(0, half, dtype=np.float64) * 2.0 / DH))
    freqs = np.arange(L, dtype=np.float64)[:, None] * inv_freq[None, :]  # [L, half]
    emb = np.concatenate([freqs, freqs], axis=-1)  # [L, DH]
    cosT = np.ascontiguousarray(np.cos(emb).T.astype(np.float32))  # [DH, L]
    sinT = np.sin(emb).T.astype(np.float32)
    sinTs = np.concatenate([-sinT[:half], sinT[half:]], axis=0)
    sinTs = np.ascontiguousarray(sinTs.astype(np.float32))
    tri = np.triu(np.ones((128, 128), dtype=np.float32))  # keep k' <= q'
    maskT = np.ascontiguousarray(np.concatenate([tri, tri], axis=1))
    ones = np.ones((128, 128), dtype=np.float32)
    return cosT, sinTs, maskT, ones


_NC_CACHE = []


def kernel(x, Wq, Wk, Wv, Wl, bl, Wo):
    x = np.asarray(x, dtype=np.float32)
    Wq = np.asarray(Wq, dtype=np.float32)
    Wk = np.asarray(Wk, dtype=np.float32)
    Wv = np.asarray(Wv, dtype=np.float32)
    Wl = np.asarray(Wl, dtype=np.float32)
    bl = np.asarray(bl, dtype=np.float32)
    Wo = np.asarray(Wo, dtype=np.float32)

    cosT, sinTs, maskT, ones = _host_tables()
    Wq3 = Wq.reshape(D, 2 * NH, DH)
    Wk3 = Wk.reshape(D, NH, DH)

    in_maps = []
    for c in range(8):
        b, g = divmod(c, G)
        wq_s = Wq3[:, 8 * g:8 * g + NQ, :].reshape(D, NQ * DH)
        wk_s = Wk3[:, G * g:G * g + NKV, :].reshape(D, NKV * DH)
        in_maps.append({
            "xT": np.ascontiguousarray(x[b].T).astype(NP_P1),
            "Wqk": np.ascontiguousarray(np.concatenate([wq_s, wk_s], axis=1)).astype(NP_P1),
            "Wv": np.ascontiguousarray(Wv[:, DH * G * g:DH * G * g + NKV * DH]).astype(NP_P1),
            "Wl": np.ascontiguousarray(np.pad(Wl[:, G * g:G * g + NKV], ((0, 0), (0, 128 - NKV)))).astype(NP_P1),
            "blv": np.ascontiguousarray(np.pad(bl[G * g:G * g + NKV], (0, 128 - NKV)).reshape(128, 1)),
            "Wo": np.ascontiguousarray(Wo[512 * g:512 * (g + 1), :]).astype(NP_P3),
            "cosT": cosT,
            "sinTs": sinTs,
            "maskT": maskT.astype(NP_ATT),
            "onesin": ones,
        })

    if not _NC_CACHE:
        _NC_CACHE.append(build_kernel())
    nc = _NC_CACHE[0]
    res = run_bass_kernel_spmd(nc, in_maps, core_ids=list(range(8)))

    out = np.empty((B, L, D), dtype=np.float32)
    for b in range(B):
        acc = res.results[4 * b]["outT"].copy()
        for g in range(1, G):
            acc += res.results[4 * b + g]["outT"]
        out[b] = acc.T
    return out



# revision 5
# speedup vs baseline: 1.0446x; 1.0446x over previous
"""DifferentialCausalAttention on 8 Trainium2 NeuronCores.

Sharding: 8 cores = 2 batches x 4 head-groups (tensor-parallel over heads).
Core c handles batch b = c // 4 and head-group g = c % 4:
  - query heads 8g..8g+7 (4 pairs), kv heads 4g..4g+3, lambda cols 4g..4g+3
  - W_O rows 512g..512g+511 -> partial output, host-summed over the 4 groups.

v2: Q/K/V + diffT stay SBUF-resident across phases (no DRAM round-trip).
Phase 1 writes RoPE'd Q directly in phase-2 superblock layout
[128(dh), pair, qtile, head, 128(l)], K as [128(dh), head, L], V as
[128(l), head, kchunk, 128(d)]. RoPE rotate-half is fused into the DVE
muls (no scalar-engine copies). Output DMA'd as bf16, host sums in fp32.
Attention (phase 2) is computed transposed (S^T = [k, q]) so no
P-transposes are needed; rowsum via ones-matmul; normalization tail
deferred one superblock so the PE never waits on ACT/DVE.
"""
import os
from contextlib import ExitStack

import ml_dtypes
import numpy as np

import concourse.bass as bass
import concourse.mybir as mybir
import concourse.tile as tile
from concourse import bacc
from concourse.bass_utils import run_bass_kernel_spmd

F32 = mybir.dt.float32
F32R = mybir.dt.float32r
BF16 = mybir.dt.bfloat16

DT = BF16
NPDT = ml_dtypes.bfloat16

B, L, D, NH = 2, 2048, 2048, 16
DH = D // NH            # 128
G = 4                   # head groups (cores per batch)
NKV = NH // G           # kv heads per core = 4
NQ = 2 * NKV            # query heads per core = 8
CQK = NQ * DH + NKV * DH  # 1536 projection cols (Q then K)
CT = CQK // 128         # 12 column tiles (0-7 Q heads, 8-11 K heads)
DC = D // 128           # 16 contraction chunks
LCH = L // 512          # 4 L-chunks
LT = L // 128           # 16 L-tiles / q-tiles
SCALE = 1.0 / float(np.sqrt(DH))
ROPE_BASE = 10000.0


def build_kernel() -> bacc.Bacc:
    nc = bacc.Bacc("TRN2", target_bir_lowering=False, debug=False)

    xT = nc.dram_tensor("xT", [D, L], DT, kind="ExternalInput")
    Wqk = nc.dram_tensor("Wqk", [D, CQK], DT, kind="ExternalInput")
    Wv = nc.dram_tensor("Wv", [D, NKV * DH], DT, kind="ExternalInput")
    Wl = nc.dram_tensor("Wl", [D, 128], DT, kind="ExternalInput")
    blv = nc.dram_tensor("blv", [128, 1], F32, kind="ExternalInput")
    Wo = nc.dram_tensor("Wo", [NKV * DH, D], DT, kind="ExternalInput")
    cosT = nc.dram_tensor("cosT", [DH, L], F32, kind="ExternalInput")
    sinTs = nc.dram_tensor("sinTs", [DH, L], F32, kind="ExternalInput")
    maskT = nc.dram_tensor("maskT", [128, 256], DT, kind="ExternalInput")
    onesin = nc.dram_tensor("onesin", [128, 128], F32R, kind="ExternalInput")
    outT = nc.dram_tensor("outT", [D, L], DT, kind="ExternalOutput")

    with ExitStack() as ctx:
        tc = ctx.enter_context(tile.TileContext(nc))

        persist = ctx.enter_context(tc.tile_pool(name="persist", bufs=1))

        # ---- persistent SBUF-resident tensors ----
        qk_q = persist.tile([128, NKV, LT, 2, 128], DT)   # Q^T, phase-2 layout
        k_all = persist.tile([128, NKV, L], DT)           # K^T per kv head
        v_all = persist.tile([128, NKV, LT, 128], DT)     # V[l, h, d]
        mask_sb = persist.tile([128, 256], DT)
        ones_sb = persist.tile([128, 128], F32R)
        ones_att = persist.tile([128, 1], DT)
        bl_sb = persist.tile([128, 1], F32)
        wo_sb = persist.tile([128, NKV, D], DT)
        lam_sb = persist.tile([NKV, L], F32)          # sigmoid(x@Wl+bl)
        diffT = persist.tile([128, NKV, L], DT)       # (ctx0 - lam*ctx1)^T

        # ================= Phase 1: projections + RoPE =================
        with tc.tile_pool(name="ph1", bufs=1) as ph1, \
                tc.tile_pool(name="ps1", bufs=1, space="PSUM") as ps1:
            xTr = xT.rearrange("(dc p) l -> p dc l", p=128)
            wqkr = Wqk.rearrange("(dc p) c -> p dc c", p=128)
            wv_sb = ph1.tile([128, DC, NKV * DH], DT)
            wl_sb = ph1.tile([128, DC, 128], DT)

            for lch in range(LCH):
                ls = slice(lch * 512, (lch + 1) * 512)
                xs = ph1.tile([128, DC, 512], DT, name="xs", tag="xs", bufs=2)
                for dc in range(DC):
                    nc.sync.dma_start(xs[:, dc, :], xTr[:, dc, ls])
                cos_sb = ph1.tile([128, 512], F32, name="cos_sb", tag="cos", bufs=2)
                sin_sb = ph1.tile([128, 512], F32, name="sin_sb", tag="sin", bufs=2)
                nc.sync.dma_start(cos_sb[:], cosT[:, ls])
                nc.sync.dma_start(sin_sb[:], sinTs[:, ls])

                # --- Q^T / K^T column tiles + RoPE ---
                for ct in range(CT):
                    wt = ph1.tile([128, DC, 128], DT, name="wt", tag="wt", bufs=3)
                    nc.sync.dma_start(wt[:], wqkr[:, :, ct * 128:(ct + 1) * 128])
                    qk_ps = ps1.tile([128, 512], F32, name="qk_ps", tag="mm512", bufs=4)
                    for dc in range(DC):
                        nc.tensor.matmul(
                            qk_ps[:], wt[:, dc, :], xs[:, dc, :],
                            start=(dc == 0), stop=(dc == DC - 1),
                        )
                    # RoPE fused: out = qk*cos + rot(qk)*sin_signed, written
                    # straight into the resident Q/K layout (bf16).
                    t1 = ph1.tile([128, 512], F32, name="t1", tag="t1", bufs=2)
                    nc.vector.tensor_mul(t1[:], qk_ps[:], cos_sb[:])
                    t2 = ph1.tile([128, 512], F32, name="t2", tag="t2", bufs=2)
                    nc.vector.tensor_mul(t2[0:64, :], qk_ps[64:128, :], sin_sb[0:64, :])
                    nc.vector.tensor_mul(t2[64:128, :], qk_ps[0:64, :], sin_sb[64:128, :])
                    if ct < NQ:
                        dest = qk_q[:, ct // 2, lch * 4:(lch + 1) * 4, ct % 2, :]
                    else:
                        dest = k_all[:, ct - NQ, ls]
                    nc.vector.tensor_add(dest, t1[:], t2[:])
                    if lch == 0 and ct == 0:
                        # big/aux loads ride behind the first column tile
                        nc.sync.dma_start(
                            wv_sb[:], Wv.rearrange("(dc p) c -> p dc c", p=128)
                        )
                        nc.sync.dma_start(
                            wl_sb[:], Wl.rearrange("(dc p) c -> p dc c", p=128)
                        )
                        nc.sync.dma_start(bl_sb[:], blv[:, :])
                        nc.sync.dma_start(mask_sb[:], maskT[:, :])
                        nc.sync.dma_start(ones_sb[:], onesin[:, :])
                        nc.vector.tensor_copy(ones_att[:], ones_sb[:, 0:1])

                # --- V tiles ---
                for lt in range(4):
                    v_ps = ps1.tile([128, 512], F32, name="v_ps", tag="mm512", bufs=4)
                    for dc in range(DC):
                        nc.tensor.matmul(
                            v_ps[:], xs[:, dc, lt * 128:(lt + 1) * 128], wv_sb[:, dc, :],
                            start=(dc == 0), stop=(dc == DC - 1),
                        )
                    nc.vector.tensor_copy(
                        v_all[:, :, lch * 4 + lt, :],
                        v_ps.rearrange("p (h d) -> p h d", h=NKV),
                    )

                # --- lambda ---
                lam_ps = ps1.tile([128, 512], F32, name="lam_ps", tag="mm512", bufs=4)
                for dc in range(DC):
                    nc.tensor.matmul(
                        lam_ps[:], wl_sb[:, dc, :], xs[:, dc, :],
                        start=(dc == 0), stop=(dc == DC - 1),
                    )
                nc.scalar.activation(
                    lam_sb[:, ls], lam_ps[0:NKV, :],
                    mybir.ActivationFunctionType.Sigmoid, bias=bl_sb[0:NKV, 0:1],
                )

        # ================= Phase 2: causal attention per head pair =================
        # Two q-tiles (a "superblock": A=2sb, B=2sb+1) are processed at once so
        # every moving operand is 512 wide: columns ordered (qtile, head, l) =
        # [A.h0 | A.h1 | B.h0 | B.h1]. Per k-chunk: one S matmul [128,512], one
        # exp, one ctx matmul, one rowsum matmul. Chunk kc==A is full width but
        # masks its [0:256] half; chunk kc==B covers only [256:512]. ctx/rs for
        # chunk kc are emitted after the S matmul of chunk kc+2 (PE never waits
        # on ACT); the normalization tail is deferred into the next superblock.
        with tc.tile_pool(name="ph2", bufs=1) as ph2, \
                tc.tile_pool(name="ps2", bufs=1, space="PSUM") as ps2:
            pend_norm = []

            def emit_block(st):
                ctx_ps, rs_ps, e_sb, j, kc, qtB, off, wid = st
                nc.tensor.matmul(
                    ctx_ps[:, off:off + wid], v_all[:, p, kc, :], e_sb[:, j, off:off + wid],
                    start=(kc == 0), stop=(kc == qtB), skip_group_check=True,
                )
                nc.tensor.matmul(
                    rs_ps[0:1, off:off + wid], ones_att[:, 0:1], e_sb[:, j, off:off + wid],
                    start=(kc == 0), stop=(kc == qtB), skip_group_check=True,
                )

            def emit_norm(st):
                ctx_ps, rs_sb, qtA, p_, lam0_ = st
                recip = ph2.tile([1, 2, 256], F32, name="recip", tag="recip", bufs=2)
                nc.vector.reciprocal_approx_fast(
                    recip.rearrange("p t l -> p (t l)"), rs_sb[:]
                )
                cs = ph2.tile([1, 2, 256], F32R, name="cs", tag="cs", bufs=2)
                nc.vector.tensor_copy(cs[:, :, 0:128], recip[:, :, 0:128])
                nc.vector.tensor_mul(
                    cs[:, :, 128:256], recip[:, :, 128:256],
                    lam0_[:, qtA * 128:(qtA + 2) * 128].rearrange(
                        "p (t l) -> p t l", t=2
                    ),
                )
                b_ps = ps2.tile([128, 512], F32, name="b_ps", tag="bps", bufs=1)
                nc.tensor.matmul(
                    b_ps[:], ones_sb[0:1, :], cs.rearrange("p t l -> p (t l)"),
                    start=True, stop=True,
                )
                b_sb = ph2.tile([128, 2, 256], F32, name="b_sb", tag="bsb", bufs=2)
                nc.vector.tensor_copy(b_sb.rearrange("p t l -> p (t l)"), b_ps[:])
                ctx3 = ctx_ps.rearrange("p (t l) -> p t l", t=2)
                t0 = ph2.tile([128, 2, 128], F32, name="t0", tag="t0", bufs=2)
                nc.vector.tensor_mul(t0[:], ctx3[:, :, 0:128], b_sb[:, :, 0:128])
                t1b = ph2.tile([128, 2, 128], F32, name="t1b", tag="t1b", bufs=2)
                nc.vector.tensor_mul(t1b[:], ctx3[:, :, 128:256], b_sb[:, :, 128:256])
                nc.vector.tensor_sub(
                    diffT[:, p_, qtA * 128:(qtA + 2) * 128],
                    t0.rearrange("p t l -> p (t l)"),
                    t1b.rearrange("p t l -> p (t l)"),
                )

            nc.sync.dma_start(wo_sb[:], Wo.rearrange("(p d) o -> d p o", d=128))
            for p in range(NKV):
                lam0 = ph2.tile([1, L], F32, name="lam0", tag="lam0", bufs=2)
                nc.gpsimd.dma_start(lam0[:], lam_sb[p:p + 1, :])

                for sb in range(LT // 2):
                    qtA, qtB = 2 * sb, 2 * sb + 1
                    ctx_ps = ps2.tile([128, 512], F32, name="ctx_ps", tag="ctx", bufs=2)
                    rs_ps = ps2.tile([1, 512], F32, name="rs_ps", tag="small", bufs=1)
                    pend = []
                    # chunk groups of 2 sharing one 2-bank S tile; exp per group
                    groups = []
                    kcs = list(range(qtB + 1))
                    for gi in range(0, len(kcs), 2):
                        groups.append(kcs[gi:gi + 2])
                    for gk, grp in enumerate(groups):
                        s_ps = ps2.tile([128, 2, 512], F32, name="s_ps", tag="s2", bufs=2)
                        segs = []
                        for j, kc in enumerate(grp):
                            off, wid = (256, 256) if kc == qtB else (0, 512)
                            rhs = (qk_q[:, p, qtA:qtA + 2, :, :] if wid == 512
                                   else qk_q[:, p, qtB, :, :])
                            nc.tensor.matmul(
                                s_ps[:, j, off:off + wid],
                                k_all[:, p, kc * 128:(kc + 1) * 128],
                                rhs,
                                start=True, stop=True, skip_group_check=True,
                            )
                            segs.append((j, kc, off, wid))
                        if gk == 0 and len(pend_norm) >= 1:
                            emit_norm(pend_norm.pop(0))  # deferred tail
                        while len(pend) >= 2:
                            emit_block(pend.pop(0))
                        e_sb = ph2.tile([128, 2, 512], DT, name="e_sb", tag="e", bufs=3)
                        if len(segs) == 2 and segs[0][3] == 512 and segs[1][3] == 512:
                            nc.scalar.activation(
                                e_sb.rearrange("p a b -> p (a b)"),
                                s_ps.rearrange("p a b -> p (a b)"),
                                mybir.ActivationFunctionType.Exp, scale=SCALE,
                            )
                        else:
                            for j, kc, off, wid in segs:
                                nc.scalar.activation(
                                    e_sb[:, j, off:off + wid], s_ps[:, j, off:off + wid],
                                    mybir.ActivationFunctionType.Exp, scale=SCALE,
                                )
                        for j, kc, off, wid in segs:
                            if kc == qtA:
                                nc.vector.tensor_mul(
                                    e_sb[:, j, 0:256], e_sb[:, j, 0:256], mask_sb[:]
                                )
                            elif kc == qtB:
                                nc.vector.tensor_mul(
                                    e_sb[:, j, 256:512], e_sb[:, j, 256:512], mask_sb[:]
                                )
                            pend.append((ctx_ps, rs_ps, e_sb, j, kc, qtB, off, wid))
                    for st in pend:
                        emit_block(st)
                    # eager rowsum copy frees the PSUM bank promptly (DVE)
                    rs_sb = ph2.tile([1, 512], F32, name="rs_sb", tag="rs_sb", bufs=3)
                    nc.vector.tensor_copy(rs_sb[:], rs_ps[:])
                    pend_norm.append((ctx_ps, rs_sb, qtA, p, lam0))
            for st in pend_norm:
                emit_norm(st)

        # ================= Phase 3: output projection =================
        with tc.tile_pool(name="ph3", bufs=1) as ph3, \
                tc.tile_pool(name="ps3", bufs=1, space="PSUM") as ps3:
            for ot in range(LT):
                for qch in range(LCH):
                    o_ps = ps3.tile([128, 512], F32, name="o_ps", tag="mm512", bufs=4)
                    for p in range(NKV):
                        nc.tensor.matmul(
                            o_ps[:],
                            wo_sb[:, p, ot * 128:(ot + 1) * 128],
                            diffT[:, p, qch * 512:(qch + 1) * 512],
                            start=(p == 0), stop=(p == NKV - 1),
                        )
                    o_sb = ph3.tile([128, 512], DT, name="o_sb", tag="osb", bufs=4)
                    nc.scalar.copy(o_sb[:], o_ps[:])
                    nc.sync.dma_start(
                        outT[ot * 128:(ot + 1) * 128, qch * 512:(qch + 1) * 512], o_sb[:]
                    )

    nc.finalize()
    return nc


def _host_tables():
    half = DH // 2
    inv_freq = 1.0 / (ROPE_BASE ** (np.arange(0, half, dtype=np.float64) * 2.0 / DH))
    freqs = np.arange(L, dtype=np.float64)[:, None] * inv_freq[None, :]  # [L, half]
    emb = np.concatenate([freqs, freqs], axis=-1)  # [L, DH]
    cosT = np.ascontiguousarray(np.cos(emb).T.astype(np.float32))  # [DH, L]
    sinT = np.sin(emb).T.astype(np.float32)
    sinTs = np.concatenate([-sinT[:half], sinT[half:]], axis=0)
    sinTs = np.ascontiguousarray(sinTs.astype(np.float32))
    tri = np.triu(np.ones((128, 128), dtype=np.float32))  # keep k' <= q'
    maskT = np.ascontiguousarray(np.concatenate([tri, tri], axis=1))
    return cosT, sinTs, maskT


_NC_CACHE = []


def kernel(x, Wq, Wk, Wv, Wl, bl, Wo):
    x = np.asarray(x, dtype=np.float32)
    Wq = np.asarray(Wq, dtype=np.float32)
    Wk = np.asarray(Wk, dtype=np.float32)
    Wv = np.asarray(Wv, dtype=np.float32)
    Wl = np.asarray(Wl, dtype=np.float32)
    bl = np.asarray(bl, dtype=np.float32)
    Wo = np.asarray(Wo, dtype=np.float32)

    cosT, sinTs, maskT = _host_tables()
    Wq3 = Wq.reshape(D, 2 * NH, DH)
    Wk3 = Wk.reshape(D, NH, DH)

    in_maps = []
    for c in range(8):
        b, g = divmod(c, G)
        wq_s = Wq3[:, 8 * g:8 * g + NQ, :].reshape(D, NQ * DH)
        wk_s = Wk3[:, G * g:G * g + NKV, :].reshape(D, NKV * DH)
        in_maps.append({
            "xT": np.ascontiguousarray(x[b].T).astype(NPDT),
            "Wqk": np.ascontiguousarray(np.concatenate([wq_s, wk_s], axis=1)).astype(NPDT),
            "Wv": np.ascontiguousarray(Wv[:, DH * G * g:DH * G * g + NKV * DH]).astype(NPDT),
            "Wl": np.ascontiguousarray(np.pad(Wl[:, G * g:G * g + NKV], ((0, 0), (0, 128 - NKV)))).astype(NPDT),
            "blv": np.ascontiguousarray(np.pad(bl[G * g:G * g + NKV], (0, 128 - NKV)).reshape(128, 1)),
            "Wo": np.ascontiguousarray(Wo[512 * g:512 * (g + 1), :]).astype(NPDT),
            "cosT": cosT,
            "sinTs": sinTs,
            "maskT": maskT.astype(NPDT),
            "onesin": np.ones((128, 128), dtype=np.float32),
        })

    if not _NC_CACHE:
        _NC_CACHE.append(build_kernel())
    nc = _NC_CACHE[0]
    res = run_bass_kernel_spmd(nc, in_maps, core_ids=list(range(8)))

    out = np.empty((B, L, D), dtype=np.float32)
    for b in range(B):
        acc = res.results[4 * b]["outT"].astype(np.float32)
        for g in range(1, G):
            acc += res.results[4 * b + g]["outT"].astype(np.float32)
        out[b] = acc.T
    return out


# revision 30
# speedup vs baseline: 1.1693x; 1.1194x over previous
"""DifferentialCausalAttention on 8 Trainium2 NeuronCores.

Sharding: 8 cores = 2 batches x 4 head-groups (tensor-parallel over heads).
Core c handles batch b = c // 4 and head-group g = c % 4:
  - query heads 8g..8g+7 (4 pairs), kv heads 4g..4g+3, lambda cols 4g..4g+3
  - W_O rows 512g..512g+511 -> partial output, host-summed over the 4 groups.

v3: Q/K/V + diffT SBUF-resident end-to-end (no DRAM round-trip). Inputs are
host-pre-laid-out so every load is fully contiguous per partition; loads are
split across both HWDGE queues (sync + scalar). Phase-2 rowsums are packed
4-at-a-time onto disjoint 32-column groups of the PE array (tile_position) so
they stream concurrently; a single bf16 selector-matmul then sums the four
partials AND broadcasts the result to all 128 partitions in one pass. Lambda
is partition-broadcast once per head on the idle GPSIMD engine. Exp always
runs as one merged [*,1024] ACT call per chunk-pair (junk columns of the
diagonal chunk are exp'd but never consumed).
"""
import os
from contextlib import ExitStack

import ml_dtypes
import numpy as np

import concourse.bass as bass
import concourse.mybir as mybir
import concourse.tile as tile
from concourse import bacc
from concourse.bass_utils import run_bass_kernel_spmd

F32 = mybir.dt.float32
F32R = mybir.dt.float32r
BF16 = mybir.dt.bfloat16

DT = BF16
NPDT = ml_dtypes.bfloat16

B, L, D, NH = 2, 2048, 2048, 16
DH = D // NH            # 128
G = 4                   # head groups (cores per batch)
NKV = NH // G           # kv heads per core = 4
NQ = 2 * NKV            # query heads per core = 8
CQK = NQ * DH + NKV * DH  # 1536 projection cols (Q then K)
CT = CQK // 128         # 12 column tiles (0-7 Q heads, 8-11 K heads)
DC = D // 128           # 16 contraction chunks
LCH = L // 512          # 4 L-chunks
LT = L // 128           # 16 L-tiles / q-tiles
SCALE = 1.0 / float(np.sqrt(DH))
ROPE_BASE = 10000.0


def build_kernel() -> bacc.Bacc:
    nc = bacc.Bacc("TRN2", target_bir_lowering=False, debug=False)

    xs_d = nc.dram_tensor("xs_d", [LCH, 128, DC * 512], DT, kind="ExternalInput")
    wqk_d = nc.dram_tensor("wqk_d", [CT, 128, DC * 128], DT, kind="ExternalInput")
    wv_d = nc.dram_tensor("wv_d", [128, DC * 512], DT, kind="ExternalInput")
    wl_d = nc.dram_tensor("wl_d", [128, DC * 128], DT, kind="ExternalInput")
    blv = nc.dram_tensor("blv", [128, 1], F32, kind="ExternalInput")
    Wo = nc.dram_tensor("Wo", [NKV * DH, D], DT, kind="ExternalInput")
    cosT = nc.dram_tensor("cosT", [DH, L], F32, kind="ExternalInput")
    sinTs = nc.dram_tensor("sinTs", [DH, L], F32, kind="ExternalInput")
    maskT = nc.dram_tensor("maskT", [128, 256], DT, kind="ExternalInput")
    selm_d = nc.dram_tensor("selm_d", [128, 512], DT, kind="ExternalInput")
    outT = nc.dram_tensor("outT", [D, L], DT, kind="ExternalOutput")
    dbg = None
    if os.environ.get("KDBG") == "1":
        dbg = {
            "lam": nc.dram_tensor("dbg_lam", [NKV, L], DT, kind="ExternalOutput"),
            "rs": nc.dram_tensor("dbg_rs", [32, 512], F32, kind="ExternalOutput"),
            "diff": nc.dram_tensor("dbg_diff", [128, NKV * L], DT, kind="ExternalOutput"),
            "qk": nc.dram_tensor("dbg_qk", [128, 512], DT, kind="ExternalOutput"),
            "v": nc.dram_tensor("dbg_v", [128, 512], DT, kind="ExternalOutput"),
        }

    with ExitStack() as ctx:
        tc = ctx.enter_context(tile.TileContext(nc))

        persist = ctx.enter_context(tc.tile_pool(name="persist", bufs=1))

        # ---- persistent SBUF-resident tensors ----
        qk_q = persist.tile([128, NKV, LT, 2, 128], DT)   # Q^T, phase-2 layout
        k_all = persist.tile([128, NKV, L], DT)           # K^T per kv head
        v_all = persist.tile([128, NKV, LT, 128], DT)     # V[l, h, d]
        mask_sb = persist.tile([128, 256], DT)
        # cols 0:128 sel4, 128:256 sel3, 256:384 sel1, col 384 all-ones
        selm = persist.tile([128, 512], DT)
        ones_att = persist.tile([128, 1], DT)
        bl_sb = persist.tile([128, 1], F32)
        wo_sb = persist.tile([128, NKV, D], DT)
        lam_sb = persist.tile([NKV, L], DT)           # sigmoid(x@Wl+bl)
        diffT = persist.tile([128, NKV, L], DT)       # (ctx0 - lam*ctx1)^T

        # ================= Phase 1: projections + RoPE =================
        with tc.tile_pool(name="ph1", bufs=1) as ph1, \
                tc.tile_pool(name="ps1", bufs=1, space="PSUM") as ps1:
            wv_sb = ph1.tile([128, DC, 512], DT)
            wl_sb = ph1.tile([128, DC, 128], DT)

            for lch in range(LCH):
                ls = slice(lch * 512, (lch + 1) * 512)
                xs = ph1.tile([128, DC, 512], DT, name="xs", tag="xs", bufs=2)
                for i in range(4):
                    nc.sync.dma_start(
                        xs[:, 4 * i:4 * (i + 1), :],
                        xs_d[lch, :, 2048 * i:2048 * (i + 1)].rearrange(
                            "p (dc c) -> p dc c", c=512),
                    )
                cos_sb = ph1.tile([128, 512], F32, name="cos_sb", tag="cos", bufs=2)
                sin_sb = ph1.tile([128, 512], F32, name="sin_sb", tag="sin", bufs=2)
                nc.sync.dma_start(cos_sb[:], cosT[:, ls])
                nc.sync.dma_start(sin_sb[:], sinTs[:, ls])

                # --- Q^T / K^T column tiles + RoPE ---
                for ct in range(CT):
                    wt = ph1.tile([128, DC, 128], DT, name="wt", tag="wt", bufs=3)
                    for i in range(2):
                        nc.scalar.dma_start(
                            wt[:, 8 * i:8 * (i + 1), :],
                            wqk_d[ct, :, 1024 * i:1024 * (i + 1)].rearrange(
                                "p (dc c) -> p dc c", c=128),
                        )
                    qk_ps = ps1.tile([128, 512], F32, name="qk_ps", tag="mm512", bufs=4)
                    for dc in range(DC):
                        nc.tensor.matmul(
                            qk_ps[:], wt[:, dc, :], xs[:, dc, :],
                            start=(dc == 0), stop=(dc == DC - 1),
                        )
                    # RoPE fused: out = qk*cos + rot(qk)*sin_signed, written
                    # straight into the resident Q/K layout (bf16).
                    t1 = ph1.tile([128, 512], F32, name="t1", tag="t1", bufs=2)
                    nc.vector.tensor_mul(t1[:], qk_ps[:], cos_sb[:])
                    t2 = ph1.tile([128, 512], F32, name="t2", tag="t2", bufs=2)
                    nc.vector.tensor_mul(t2[0:64, :], qk_ps[64:128, :], sin_sb[0:64, :])
                    nc.vector.tensor_mul(t2[64:128, :], qk_ps[0:64, :], sin_sb[64:128, :])
                    if ct < NQ:
                        dest = qk_q[:, ct // 2, lch * 4:(lch + 1) * 4, ct % 2, :]
                    else:
                        dest = k_all[:, ct - NQ, ls]
                    nc.vector.tensor_add(dest, t1[:], t2[:])
                    if lch == 0 and ct == 0:
                        # big/aux loads ride behind the first column tile
                        nc.scalar.dma_start(
                            wv_sb[:], wv_d.rearrange("p (dc c) -> p dc c", c=512)
                        )
                        nc.scalar.dma_start(
                            wl_sb[:], wl_d.rearrange("p (dc c) -> p dc c", c=128)
                        )
                        nc.sync.dma_start(bl_sb[:], blv[:, :])
                        nc.sync.dma_start(mask_sb[:], maskT[:, :])
                        nc.sync.dma_start(selm[:], selm_d[:, :])
                        nc.vector.tensor_copy(ones_att[:], selm[:, 384:385])

                # --- V tiles ---
                for lt in range(4):
                    v_ps = ps1.tile([128, 512], F32, name="v_ps", tag="mm512", bufs=4)
                    for dc in range(DC):
                        nc.tensor.matmul(
                            v_ps[:], xs[:, dc, lt * 128:(lt + 1) * 128], wv_sb[:, dc, :],
                            start=(dc == 0), stop=(dc == DC - 1),
                        )
                    nc.vector.tensor_copy(
                        v_all[:, :, lch * 4 + lt, :],
                        v_ps.rearrange("p (h d) -> p h d", h=NKV),
                    )

                # --- lambda ---
                lam_ps = ps1.tile([128, 512], F32, name="lam_ps", tag="mm512", bufs=4)
                for dc in range(DC):
                    nc.tensor.matmul(
                        lam_ps[:], wl_sb[:, dc, :], xs[:, dc, :],
                        start=(dc == 0), stop=(dc == DC - 1),
                    )
                nc.scalar.activation(
                    lam_sb[:, ls], lam_ps[0:NKV, :],
                    mybir.ActivationFunctionType.Sigmoid, bias=bl_sb[0:NKV, 0:1],
                )

        # ================= Phase 2: causal attention per head pair =================
        # Two q-tiles (a "superblock": A=2sb, B=2sb+1) are processed at once so
        # every moving operand is 512 wide: columns ordered (qtile, head, l) =
        # [A.h0 | A.h1 | B.h0 | B.h1]. Per k-chunk: one S matmul [128,512], one
        # merged exp per chunk-pair, one ctx matmul; rowsums are deferred into
        # bursts of 4 on disjoint PE column groups (strips 0/32/64/96 of a
        # single PSUM bank, one has_written clear per superblock) so the four
        # 1-row matmuls stream concurrently. A bf16 selector matmul then sums
        # the strips AND broadcasts the total to 128 partitions in one pass.
        # The normalization tail is deferred into the next superblock.
        with tc.tile_pool(name="ph2", bufs=1) as ph2, \
                tc.tile_pool(name="ps2", bufs=1, space="PSUM") as ps2:
            pend_norm = []

            def emit_ctx(st):
                ctx_ps, e_sb, j, kc, qtB, off, wid = st
                nc.tensor.matmul(
                    ctx_ps[:, off:off + wid], v_all[:, p, kc, :], e_sb[:, j, off:off + wid],
                    start=(kc == 0), stop=(kc == qtB), skip_group_check=True,
                )

            def emit_rs_burst(rs_ps, burst):
                for (e_sb, j, kc, off, wid, strip, is_last) in burst:
                    if kc == 0:
                        # M=128 with lhsT [ones | 0...]: partition 0 gets the
                        # rowsum, partitions 1-127 get 0 — zero-initializes
                        # the whole bank in the same streaming pass so later
                        # strip matmuls accumulate onto clean zeros.
                        nc.tensor.matmul(
                            rs_ps[:, :], selm[:, 384:512], e_sb[:, j, :],
                            start=True, stop=is_last, skip_group_check=True,
                        )
                    else:
                        nc.tensor.matmul(
                            rs_ps[32 * strip:32 * strip + 1, off:off + wid],
                            ones_att[:, 0:1], e_sb[:, j, off:off + wid],
                            start=False, stop=is_last, skip_group_check=True,
                            tile_position=(0, 32 * strip),
                        )

            def emit_norm(st):
                ctx_ps, rs4_sb, qtA, p_, lamB_, sbid = st
                # sum the 4 strips + broadcast to 128 partitions in one matmul
                b_ps = ps2.tile([128, 512], F32, name="b_ps", tag="bps", bufs=1)
                nc.tensor.matmul(
                    b_ps[:], selm[:, 0:128], rs4_sb[:],
                    start=True, stop=True,
                )
                if dbg is not None:
                    dtile = ph2.tile([1, 512], F32, name="dtile", tag="dt", bufs=2)
                    nc.vector.tensor_copy(dtile[:], b_ps[0:1, :])
                    nc.sync.dma_start(dbg["rs"][sbid:sbid + 1, :], dtile[:])
                binv = ph2.tile([128, 2, 256], F32, name="binv", tag="binv", bufs=2)
                nc.vector.reciprocal_approx_fast(
                    binv.rearrange("p t l -> p (t l)"), b_ps[:]
                )
                nc.vector.tensor_mul(
                    binv[:, :, 128:256], binv[:, :, 128:256],
                    lamB_[:, qtA * 128:(qtA + 2) * 128].rearrange(
                        "p (t l) -> p t l", t=2),
                )
                ctx3 = ctx_ps.rearrange("p (t l) -> p t l", t=2)
                t0 = ph2.tile([128, 2, 128], F32, name="t0", tag="t0", bufs=2)
                nc.vector.tensor_mul(t0[:], ctx3[:, :, 0:128], binv[:, :, 0:128])
                t1b = ph2.tile([128, 2, 128], F32, name="t1b", tag="t1b", bufs=2)
                nc.vector.tensor_mul(t1b[:], ctx3[:, :, 128:256], binv[:, :, 128:256])
                nc.vector.tensor_sub(
                    diffT[:, p_, qtA * 128:(qtA + 2) * 128],
                    t0.rearrange("p t l -> p (t l)"),
                    t1b.rearrange("p t l -> p (t l)"),
                )

            nc.sync.dma_start(wo_sb[:], Wo.rearrange("(p d) o -> d p o", d=128))
            for p in range(NKV):
                lam0 = ph2.tile([1, L], DT, name="lam0", tag="lam0", bufs=2)
                nc.gpsimd.dma_start(lam0[:], lam_sb[p:p + 1, :])
                lamB = ph2.tile([128, L], DT, name="lamB", tag="lamB", bufs=2)
                nc.gpsimd.partition_broadcast(lamB[:], lam0[:])

                for sb in range(LT // 2):
                    qtA, qtB = 2 * sb, 2 * sb + 1
                    ctx_ps = ps2.tile([128, 512], F32, name="ctx_ps", tag="ctx", bufs=2)
                    rs_ps = ps2.tile([128, 512], F32, name="rs_ps", tag="rs4", bufs=1)
                    pend = []
                    rs_pend = []
                    # rowsum strips: chunk kc accumulates on partition 32*(kc%4)
                    strip_of = {kc: kc % 4 for kc in range(qtB + 1)}
                    last_on = {}
                    for kc, s in strip_of.items():
                        last_on[s] = max(last_on.get(s, -1), kc)
                    groups = []
                    kcs = list(range(qtB + 1))
                    for gi in range(0, len(kcs), 2):
                        groups.append(kcs[gi:gi + 2])
                    for gk, grp in enumerate(groups):
                        s_ps = ps2.tile([128, 2, 512], F32, name="s_ps", tag="s2", bufs=2)
                        segs = []
                        for j, kc in enumerate(grp):
                            # S is always computed full width (the non-causal
                            # [0:256] of the diagonal chunk is never consumed,
                            # but writing it keeps the merged exp fully
                            # initialized); ctx/rowsum use only off:off+wid.
                            off, wid = (256, 256) if kc == qtB else (0, 512)
                            nc.tensor.matmul(
                                s_ps[:, j, :],
                                k_all[:, p, kc * 128:(kc + 1) * 128],
                                qk_q[:, p, qtA:qtA + 2, :, :],
                                start=True, stop=True, skip_group_check=True,
                            )
                            segs.append((j, kc, off, wid))
                        if gk == 0 and len(pend_norm) >= 1:
                            emit_norm(pend_norm.pop(0))  # deferred tail
                        while len(pend) >= 4:
                            emit_ctx(pend.pop(0))
                        if len(rs_pend) >= 4:
                            emit_rs_burst(rs_ps, rs_pend)
                            rs_pend = []
                        # merged exp over the whole 2-bank group; the unused
                        # [0:256] half of a diagonal chunk is exp'd junk that
                        # no consumer reads (start=True cleared the bank bits).
                        e_sb = ph2.tile([128, 2, 512], DT, name="e_sb", tag="e", bufs=4)
                        nc.scalar.activation(
                            e_sb.rearrange("p a b -> p (a b)"),
                            s_ps.rearrange("p a b -> p (a b)"),
                            mybir.ActivationFunctionType.Exp, scale=SCALE,
                        )
                        for j, kc, off, wid in segs:
                            if kc == qtA:
                                nc.vector.tensor_mul(
                                    e_sb[:, j, 0:256], e_sb[:, j, 0:256], mask_sb[:]
                                )
                            elif kc == qtB:
                                nc.vector.tensor_mul(
                                    e_sb[:, j, 256:512], e_sb[:, j, 256:512], mask_sb[:]
                                )
                            pend.append((ctx_ps, e_sb, j, kc, qtB, off, wid))
                            rs_pend.append(
                                (e_sb, j, kc, off, wid, strip_of[kc],
                                 kc == last_on[strip_of[kc]]))
                    for st in pend:
                        emit_ctx(st)
                    if rs_pend:
                        emit_rs_burst(rs_ps, rs_pend)
                    # copy the strip partials out promptly (bf16; garbage
                    # partitions are masked by the selector matmul later)
                    rs4_sb = ph2.tile([128, 512], DT, name="rs4_sb", tag="rs4sb", bufs=3)
                    nc.vector.tensor_copy(rs4_sb[:], rs_ps[:])
                    pend_norm.append((ctx_ps, rs4_sb, qtA, p, lamB, p * 8 + sb))
            for st in pend_norm:
                emit_norm(st)
            if dbg is not None:
                nc.sync.dma_start(dbg["lam"][:], lam_sb[:, :])
                nc.sync.dma_start(
                    dbg["diff"][:], diffT.rearrange("p h l -> p (h l)"))
                nc.sync.dma_start(dbg["qk"][:], qk_q[:, 0, 0:2, :, :].rearrange(
                    "p t h l -> p (t h l)"))
                nc.sync.dma_start(dbg["v"][:], v_all[:, 0, 0:4, :].rearrange(
                    "p k l -> p (k l)"))

        # ================= Phase 3: output projection =================
        # qch-outer so the first output tiles only need diffT columns that
        # were normalized long ago (the deferred tail finishes qch=3 last).
        with tc.tile_pool(name="ph3", bufs=1) as ph3, \
                tc.tile_pool(name="ps3", bufs=1, space="PSUM") as ps3:
            for qch in range(LCH):
                for ot in range(LT):
                    o_ps = ps3.tile([128, 512], F32, name="o_ps", tag="mm512", bufs=4)
                    for p in range(NKV):
                        nc.tensor.matmul(
                            o_ps[:],
                            wo_sb[:, p, ot * 128:(ot + 1) * 128],
                            diffT[:, p, qch * 512:(qch + 1) * 512],
                            start=(p == 0), stop=(p == NKV - 1),
                        )
                    o_sb = ph3.tile([128, 512], DT, name="o_sb", tag="osb", bufs=4)
                    nc.scalar.copy(o_sb[:], o_ps[:])
                    nc.sync.dma_start(
                        outT[ot * 128:(ot + 1) * 128, qch * 512:(qch + 1) * 512], o_sb[:]
                    )

    nc.finalize()
    return nc


def _host_tables():
    half = DH // 2
    inv_freq = 1.0 / (ROPE_BASE ** (np.arange(0, half, dtype=np.float64) * 2.0 / DH))
    freqs = np.arange(L, dtype=np.float64)[:, None] * inv_freq[None, :]  # [L, half]
    emb = np.concatenate([freqs, freqs], axis=-1)  # [L, DH]
    cosT = np.ascontiguousarray(np.cos(emb).T.astype(np.float32))  # [DH, L]
    sinT = np.sin(emb).T.astype(np.float32)
    sinTs = np.concatenate([-sinT[:half], sinT[half:]], axis=0)
    sinTs = np.ascontiguousarray(sinTs.astype(np.float32))
    tri = np.triu(np.ones((128, 128), dtype=np.float32))  # keep k' <= q'
    maskT = np.ascontiguousarray(np.concatenate([tri, tri], axis=1))
    selm = np.zeros((128, 512), dtype=np.float32)
    selm[(0, 32, 64, 96), 0:128] = 1.0   # sel4: sum strips 0/32/64/96
    selm[(0, 32, 64), 128:256] = 1.0     # sel3: superblock 1 (4 chunks, 3 strips)
    selm[0, 256:384] = 1.0               # sel1: superblock 0 (2 chunks, 1 strip)
    selm[:, 384] = 1.0                   # all-ones column for the rowsum lhsT
    return cosT, sinTs, maskT, selm


_NC_CACHE = []


def kernel(x, Wq, Wk, Wv, Wl, bl, Wo):
    x = np.asarray(x, dtype=np.float32)
    Wq = np.asarray(Wq, dtype=np.float32)
    Wk = np.asarray(Wk, dtype=np.float32)
    Wv = np.asarray(Wv, dtype=np.float32)
    Wl = np.asarray(Wl, dtype=np.float32)
    bl = np.asarray(bl, dtype=np.float32)
    Wo = np.asarray(Wo, dtype=np.float32)

    cosT, sinTs, maskT, selm = _host_tables()
    Wq3 = Wq.reshape(D, 2 * NH, DH)
    Wk3 = Wk.reshape(D, NH, DH)

    in_maps = []
    for c in range(8):
        b, g = divmod(c, G)
        wq_s = Wq3[:, 8 * g:8 * g + NQ, :].reshape(D, NQ * DH)
        wk_s = Wk3[:, G * g:G * g + NKV, :].reshape(D, NKV * DH)
        wqk = np.concatenate([wq_s, wk_s], axis=1)          # [D, 1536]
        # pre-layouts: fully contiguous per-partition DMA lines
        xs_pre = np.ascontiguousarray(
            x[b].T.reshape(DC, 128, LCH, 512).transpose(2, 1, 0, 3)
            .reshape(LCH, 128, DC * 512)).astype(NPDT)
        wqk_pre = np.ascontiguousarray(
            wqk.reshape(DC, 128, CT, 128).transpose(2, 1, 0, 3)
            .reshape(CT, 128, DC * 128)).astype(NPDT)
        wv_pre = np.ascontiguousarray(
            Wv[:, DH * G * g:DH * G * g + NKV * DH]
            .reshape(DC, 128, 512).transpose(1, 0, 2).reshape(128, DC * 512)
        ).astype(NPDT)
        wl_pre = np.ascontiguousarray(
            np.pad(Wl[:, G * g:G * g + NKV], ((0, 0), (0, 128 - NKV)))
            .reshape(DC, 128, 128).transpose(1, 0, 2).reshape(128, DC * 128)
        ).astype(NPDT)
        in_maps.append({
            "xs_d": xs_pre,
            "wqk_d": wqk_pre,
            "wv_d": wv_pre,
            "wl_d": wl_pre,
            "blv": np.ascontiguousarray(np.pad(bl[G * g:G * g + NKV], (0, 128 - NKV)).reshape(128, 1)),
            "Wo": np.ascontiguousarray(Wo[512 * g:512 * (g + 1), :]).astype(NPDT),
            "cosT": cosT,
            "sinTs": sinTs,
            "maskT": maskT.astype(NPDT),
            "selm_d": selm.astype(NPDT),
        })

    if not _NC_CACHE:
        _NC_CACHE.append(build_kernel())
    nc = _NC_CACHE[0]
    res = run_bass_kernel_spmd(nc, in_maps, core_ids=list(range(8)))

    out = np.empty((B, L, D), dtype=np.float32)
    for b in range(B):
        acc = res.results[4 * b]["outT"].astype(np.float32)
        for g in range(1, G):
            acc += res.results[4 * b + g]["outT"].astype(np.float32)
        out[b] = acc.T
    return out
